# revision 9
# baseline (speedup 1.0000x reference)
"""KL-divergence loss kernel (C51 categorical projection + batchmean KL) for TRN2.

Math: the reference projects `anchor` through a C51 projection whose skew is a
compile-time scalar, so the projection collapses to a constant linear map:

    t[:, 0]  = 0
    t[:, 1]  = 0.75*a[:, 0]
    t[:, j]  = 0.75*a[:, j-1] + 0.25*a[:, j-2]          (2 <= j <= 49)
    t[:, 50] = 0.25*a[:, 48] + a[:, 49] + a[:, 50]

and the loss is sum(t * (log t - log(f + 1e-16))) / B  (terms with t==0 are 0).

Kernel strategy (pure data parallel over 8 cores, batch-sharded):
  s = 4t built with one wide fused scalar_tensor_tensor (s_j = 3*a_{j-1} + a_{j-2})
  lt = Ln(s + 4e-16)   -> bf16   [ScalarE, fused affine]
  lf = Ln(4f + 4e-16)  -> bf16   [ScalarE, fused affine]  (folds the 1/4 scale of s)
  Split accumulation on TensorE (avoids a full-width VectorE subtract):
    acc[:, 0:128]   += s_blk^T @ lt_blk   (diagonal = per-column sum of s*lt)
    acc[:, 128:256] += s_blk^T @ lf_blk
  Host sums the two diagonals of the 8 per-core results:
    loss = 0.25 * (sum diag_t - sum diag_f) / B.

Schedule: DMA geometry is decoupled from compute geometry.  Inputs stream as
three R=128 slabs (26 KB contiguous per partition -> ~390 GB/s vs ~334 GB/s
for R=64 transfers) which compute consumes as R=64 half-views; the remaining
quarter of the data streams as progressively smaller tiles (R=64/32/16/8/8)
so the post-stream tail chain (stt -> Ln -> matmul) is short.  The final tile
loads f before a and computes lf before lt so the last-arriving transfer
feeds the shortest remaining chain.
"""

import os
import numpy as np

B_TOTAL = 524288
ATOMS = 51
N_CORES = 8
ROWS_PER_CORE = B_TOTAL // N_CORES  # 65536
P = 128
MM_BLOCK = 128

# (dma_R, [compute_R ...]) per segment; dma_R rows/partition are loaded in one
# transfer and computed in sub-tiles of compute_R rows/partition.
SEGMENTS = [
    (128, [64, 64]),
    (128, [64, 64]),
    (128, [64, 64]),
    (64, [64]),
    (32, [32]),
    (16, [16]),
    (8, [8]),
    (8, [8]),
]
assert sum(r for r, _ in SEGMENTS) * P == ROWS_PER_CORE
assert all(sum(cs) == r for r, cs in SEGMENTS)

_BUILT = None
_LAST_RESULTS = None


def _build():
    from contextlib import ExitStack

    import concourse.bacc as bacc
    import concourse.tile as tile
    from concourse import mybir

    nc = bacc.Bacc("TRN2", num_devices=N_CORES)

    a_dram = nc.dram_tensor(
        "anchor", [ROWS_PER_CORE, ATOMS], mybir.dt.float32, kind="ExternalInput"
    )
    f_dram = nc.dram_tensor(
        "feature", [ROWS_PER_CORE, ATOMS], mybir.dt.float32, kind="ExternalInput"
    )
    out_dram = nc.dram_tensor(
        "out", [P, 2 * MM_BLOCK], mybir.dt.float32, kind="ExternalOutput"
    )

    # Per-geometry views of the DRAM tensors: index [n] selects a tile of
    # P*R rows; a segment starting at row r0 uses index r0 // (P*R).
    a_views = {}
    f_views = {}
    for R in sorted({r for r, _ in SEGMENTS}):
        a_views[R] = a_dram.ap().rearrange("(n p q) m -> n p (q m)", p=P, q=R)
        f_views[R] = f_dram.ap().rearrange("(n p q) m -> n p (q m)", p=P, q=R)

    mult = mybir.AluOpType.mult
    add = mybir.AluOpType.add

    def blocks_for(tile_cols):
        n_full, tail = divmod(tile_cols, MM_BLOCK)
        blk = [(b * MM_BLOCK, MM_BLOCK) for b in range(n_full)]
        if tail:
            blk.append((n_full * MM_BLOCK, tail))
        return blk

    total_mms = sum(
        2 * len(blocks_for(c * ATOMS)) for _, cs in SEGMENTS for c in cs
    )

    with tile.TileContext(nc) as tc:
        with ExitStack() as ctx:
            a_pool = ctx.enter_context(tc.tile_pool(name="a", bufs=2))
            f_pool = ctx.enter_context(tc.tile_pool(name="f", bufs=2))
            s_pool = ctx.enter_context(tc.tile_pool(name="s", bufs=2))
            lt_pool = ctx.enter_context(tc.tile_pool(name="lt", bufs=2))
            lf_pool = ctx.enter_context(tc.tile_pool(name="lf", bufs=2))
            tmp_pool = ctx.enter_context(tc.tile_pool(name="tmp", bufs=2))
            out_pool = ctx.enter_context(tc.tile_pool(name="outp", bufs=1))
            psum_pool = ctx.enter_context(
                tc.tile_pool(name="acc", bufs=1, space="PSUM")
            )

            acc = psum_pool.tile([P, 2 * MM_BLOCK], mybir.dt.float32)

            eps = out_pool.tile([P, 1], mybir.dt.float32, tag="eps")
            nc.gpsimd.memset(eps[:], 4e-16)

            mm = 0
            r0 = 0
            for seg_i, (R, comp_rs) in enumerate(SEGMENTS):
                dma_cols = R * ATOMS
                idx = r0 // (P * R)
                r0 += P * R
                last = seg_i == len(SEGMENTS) - 1

                a_sb = a_pool.tile([P, dma_cols], mybir.dt.float32, tag="a")
                f_sb = f_pool.tile([P, dma_cols], mybir.dt.float32, tag="f")
                if last:
                    # f first: the final transfer (a) feeds the short
                    # stt -> lt -> mm_t chain; lf runs while a lands.
                    nc.sync.dma_start(out=f_sb[:], in_=f_views[R][idx])
                    nc.sync.dma_start(out=a_sb[:], in_=a_views[R][idx])
                else:
                    nc.sync.dma_start(out=a_sb[:], in_=a_views[R][idx])
                    nc.sync.dma_start(out=f_sb[:], in_=f_views[R][idx])

                c0r = 0
                for cR in comp_rs:
                    tile_cols = cR * ATOMS
                    av = a_sb[:, c0r : c0r + tile_cols]
                    fv = f_sb[:, c0r : c0r + tile_cols]
                    c0r += tile_cols

                    s_sb = s_pool.tile([P, tile_cols], mybir.dt.bfloat16, tag="s")
                    lt_sb = lt_pool.tile(
                        [P, tile_cols], mybir.dt.bfloat16, tag="lt"
                    )
                    lf_sb = lf_pool.tile(
                        [P, tile_cols], mybir.dt.bfloat16, tag="lf"
                    )
                    tmp = tmp_pool.tile([P, cR], mybir.dt.float32, tag="tmp")

                    a3 = av.rearrange("p (q m) -> p q m", m=ATOMS)
                    s3 = s_sb[:].rearrange("p (q m) -> p q m", m=ATOMS)

                    # s_j = 3*a_{j-1} + a_{j-2} for j in 2..49
                    nc.vector.scalar_tensor_tensor(
                        out=s3[:, :, 2:50],
                        in0=a3[:, :, 1:49],
                        scalar=3.0,
                        in1=a3[:, :, 0:48],
                        op0=mult,
                        op1=add,
                    )
                    # s_1 = 3*a_0 ; s_0 = 0
                    nc.vector.tensor_scalar_mul(s3[:, :, 1], a3[:, :, 0], 3.0)
                    nc.gpsimd.memset(s3[:, :, 0], 0.0)
                    # s_50 = a_48 + 4*a_49 + 4*a_50
                    nc.vector.scalar_tensor_tensor(
                        out=tmp[:],
                        in0=a3[:, :, 49],
                        scalar=4.0,
                        in1=a3[:, :, 48],
                        op0=mult,
                        op1=add,
                    )
                    nc.vector.scalar_tensor_tensor(
                        out=s3[:, :, 50],
                        in0=a3[:, :, 50],
                        scalar=4.0,
                        in1=tmp[:],
                        op0=mult,
                        op1=add,
                    )

                    blocks = blocks_for(tile_cols)

                    def emit_t():
                        nonlocal mm
                        # lt = Ln(s + 4e-16)
                        nc.scalar.activation(
                            out=lt_sb[:],
                            in_=s_sb[:],
                            func=mybir.ActivationFunctionType.Ln,
                            bias=eps[:],
                            scale=1.0,
                        )
                        for c0, w in blocks:
                            nc.tensor.matmul(
                                acc[0:w, 0:w],
                                s_sb[:, c0 : c0 + w],
                                lt_sb[:, c0 : c0 + w],
                                start=(mm == 0),
                                stop=(mm == total_mms - 1),
                            )
                            mm += 1

                    def emit_f():
                        nonlocal mm
                        # lf = Ln(4f + 4e-16)
                        nc.scalar.activation(
                            out=lf_sb[:],
                            in_=fv,
                            func=mybir.ActivationFunctionType.Ln,
                            bias=eps[:],
                            scale=4.0,
                        )
                        for c0, w in blocks:
                            nc.tensor.matmul(
                                acc[0:w, MM_BLOCK : MM_BLOCK + w],
                                s_sb[:, c0 : c0 + w],
                                lf_sb[:, c0 : c0 + w],
                                start=(mm == 0),
                                stop=(mm == total_mms - 1),
                            )
                            mm += 1

                    if last:
                        # f arrived first; its chain runs while a lands
                        emit_f()
                        emit_t()
                    else:
                        # lt first: its input chain (a -> stt) is longer
                        # and ScalarE runs in order
                        emit_t()
                        emit_f()

            out_sb = out_pool.tile([P, 2 * MM_BLOCK], mybir.dt.float32, tag="out")
            nc.vector.tensor_copy(out_sb[:], acc[:])
            nc.sync.dma_start(out=out_dram.ap(), in_=out_sb[:])

    nc.compile()
    return nc


def kernel(anchor: np.ndarray, feature: np.ndarray) -> np.ndarray:
    global _BUILT, _LAST_RESULTS
    from concourse import bass_utils

    if _BUILT is None:
        _BUILT = _build()
    nc = _BUILT

    anchor = np.ascontiguousarray(anchor, dtype=np.float32)
    feature = np.ascontiguousarray(feature, dtype=np.float32)

    in_maps = []
    for c in range(N_CORES):
        lo, hi = c * ROWS_PER_CORE, (c + 1) * ROWS_PER_CORE
        in_maps.append({"anchor": anchor[lo:hi], "feature": feature[lo:hi]})

    res = bass_utils.run_bass_kernel_spmd(
        nc,
        in_maps,
        core_ids=list(range(N_CORES)),
        trace=bool(os.environ.get("BASS_TRACE")),
    )
    _LAST_RESULTS = res

    total = 0.0
    for c in range(N_CORES):
        o = res.results[c]["out"].astype(np.float64)
        total += np.trace(o[:, :MM_BLOCK]) - np.trace(o[:, MM_BLOCK:])
    val = 0.25 * total / B_TOTAL
    return np.float32(val)
